# revision 1
# baseline (speedup 1.0000x reference)
"""Trainium2 Bass kernel for nn_BeBertEmbedding (self-contained).

Math: the reference's semantic_embed(ids, W, b, pad=0) is
    where(ids==0, take(W.T, ids) + b, zeros)
so the only table row that survives is W[:, 0], and the whole module is
    out[b,s,:] = pe[s,:] + (ids[b,s]==0)*(W_tok[:,0]+b_tok)
                         + (seg[b,s]==0)*(W_seg[:,0]+b_seg)

Sharding: sequence-parallel across 8 cores (256 positions/core, all 16
batches; each core writes a disjoint [16, 256, 768] slice, no collectives).

Per core the device program is raw Bass (no Tile — avoids the kernel-tail
drain/barrier):
  * one packed const tensor: [c_seg | seg-masks | (c_tok | tok-masks) |
    pe tile0 | pe tile1], loaded in two DMAs (the first covers everything
    the t=0 tiles need, so compute starts before pe tile1 lands),
  * 32 output tiles [128 tokens, 768]: one fused DVE scalar_tensor_tensor
    (c_seg * m2[p]) + pe each (plus a second op for the rare tiles with a
    zero token id), through NSLOT SBUF slots,
  * out-DMAs issued alternately from the SP and ACT sequencers — TRN2's two
    physical HW-DGE rings — keeping the stream HBM-write-bandwidth-bound,
  * per-slot semaphores (DMAs on different HW queues complete out of order,
    a single counting sem cannot tell WHICH slot freed).

Cost-model estimate 33.2us/core; streams in isolation: DVE ~29.4us, HWDGE
writes 24.0us (model; real HBM is ~358GB/s per core SHARED by reads and
writes, so the real floor is (12MB out + 0.8MB in)/358GB/s ~= 36us — this
kernel sits on that roofline).  Rejected variants kept behind env flags:
BASS_KERNEL_POOL_SPLIT (walrus: no TensorScalarPtr on Pool),
BASS_KERNEL_DEV_BCAST (on-device c_seg broadcast saves 0.4MB HBM read but
the cvec DMA + gpsimd ucode library reload + broadcast chain costs more
head than it saves).  An indirect-DMA scatter variant (2 DVE ops + 2 SWDGE
scatters, see sim_scatter.py in the dev tree) sims at 10.6us but that is a
cost-model artifact — it does not charge SWDGE data movement; real SWDGE
queue->engine spread is unknown, so it is not shipped.
"""

import contextlib
import os
import sys

import numpy as np

try:
    from concourse import bacc, bass, mybir, tile
    from concourse.bass_utils import run_bass_kernel_spmd
except ImportError:
    for _p in ("/opt/trn_rl_repo", "/root/.axon_site/_ro/trn_rl_repo"):
        if os.path.isdir(_p) and _p not in sys.path:
            sys.path.insert(0, _p)
            break
    from concourse import bacc, bass, mybir, tile
    from concourse.bass_utils import run_bass_kernel_spmd

N_CORES = 8
B, SEQ, D = 16, 2048, 768
S_SH = SEQ // N_CORES        # 256 sequence positions per core
P = 128                      # partitions
T_TILES = S_SH // P          # 2 seq tiles per core
J = B * T_TILES              # 32 output tiles per core
NSLOT = 16                   # SBUF output slots

_F32 = mybir.dt.float32

_prog_cache: dict = {}
LAST_RESULTS = None          # BassKernelResults of the most recent run


# ---------------------------------------------------------------- raw bass —

def _layout(tok_cols, dev_bcast=False):
    """Column offsets inside the packed per-core const tensor [128, C].
    With dev_bcast the c-vectors are NOT in this tensor (they arrive as a
    [1, D]-per-vector tensor and are partition-broadcast on device)."""
    if dev_bcast:
        M2_OFF = 0
        off = M2_OFF + J
        if tok_cols:
            M1_OFF = off
            off = M1_OFF + J
        else:
            M1_OFF = None
        CSEG_OFF = CTOK_OFF = None
        PE_OFF = off
        C = PE_OFF + T_TILES * D
        A_COLS = PE_OFF + D
        return CSEG_OFF, M2_OFF, CTOK_OFF, M1_OFF, PE_OFF, C, A_COLS
    CSEG_OFF = 0
    M2_OFF = CSEG_OFF + D
    off = M2_OFF + J
    if tok_cols:
        CTOK_OFF = off
        M1_OFF = CTOK_OFF + D
        off = M1_OFF + J
    else:
        CTOK_OFF = M1_OFF = None
    PE_OFF = off
    C = PE_OFF + T_TILES * D
    A_COLS = PE_OFF + D          # DMA-A: everything + pe tile 0
    return CSEG_OFF, M2_OFF, CTOK_OFF, M1_OFF, PE_OFF, C, A_COLS


def _order(i):
    """Compute-order index i -> output tile j: all t=0 tiles first (they
    only need the first const DMA), then t=1 tiles."""
    return 2 * i if i < J // 2 else 2 * (i - J // 2) + 1


def _build_raw(
    tok_cols: frozenset, pool_split: bool = False, dev_bcast: bool = False
) -> "bass.Bass":
    CSEG_OFF, M2_OFF, CTOK_OFF, M1_OFF, PE_OFF, C, A_COLS = _layout(
        tok_cols, dev_bcast
    )
    assert not (pool_split and dev_bcast)
    NV = 2 if tok_cols else 1    # number of broadcast c-vectors
    nc = bass.Bass("TRN2")
    const_d = nc.dram_tensor("consts", [P, C], _F32, kind="ExternalInput")
    if dev_bcast:
        cvec_d = nc.dram_tensor("cvec", [1, NV * D], _F32, kind="ExternalInput")
    out_d = nc.dram_tensor("out", [B * S_SH, D], _F32, kind="ExternalOutput")
    mult, add = mybir.AluOpType.mult, mybir.AluOpType.add

    # Which compute order-indices run on GPSIMD (Pool) instead of DVE.
    # i%4==3 keeps slot ownership disjoint (slots 3,7 are pool-exclusive, so
    # same-slot WAR chains never cross engines).  Tok tiles stay on DVE.
    if pool_split:
        on_pool = [i % 4 == 3 and _order(i) not in tok_cols for i in range(J)]
    else:
        on_pool = [False] * J
    # per-engine completion counts up to and including index i
    dve_cnt, pool_cnt, dc, pc = [], [], 0, 0
    for i in range(J):
        if on_pool[i]:
            pc += 1
        else:
            dc += 1
        dve_cnt.append(dc)
        pool_cnt.append(pc)

    with contextlib.ExitStack() as stack:
        c_t = stack.enter_context(nc.sbuf_tensor([P, C], _F32))
        if dev_bcast:
            cw = stack.enter_context(nc.sbuf_tensor([P, NV * D], _F32))
            cv_sem = stack.enter_context(nc.semaphore("cv_sem"))
            bc_sem = stack.enter_context(nc.semaphore("bc_sem"))
        obuf = stack.enter_context(nc.sbuf_tensor([P, NSLOT * D], _F32))
        a_sem = stack.enter_context(nc.semaphore("a_sem"))
        b_sem = stack.enter_context(nc.semaphore("b_sem"))
        v_sem = stack.enter_context(nc.semaphore("v_sem"))
        p_sem = stack.enter_context(nc.semaphore("p_sem"))
        s_sems = [
            stack.enter_context(nc.semaphore(f"slot_sem{k}")) for k in range(NSLOT)
        ]
        # DVE's pipeline is deep: the tok tile's second STT reads the first's
        # output on the same engine and needs an explicit retire guard.
        t_sem = stack.enter_context(nc.semaphore("tok_sem"))
        block = stack.enter_context(nc.Block())

        def issue_out_dmas(eng, parity):
            for i in range(J):
                if i % 2 != parity:
                    continue
                j = _order(i)
                s = i % NSLOT
                done = (p_sem, pool_cnt[i]) if on_pool[i] else (v_sem, dve_cnt[i])
                # wait attached inline: TRN2 allows exactly one wait per
                # instruction, and this saves a sequencer dispatch slot
                eng.dma_start(
                    out_d[j * P:(j + 1) * P, :], obuf[:, s * D:(s + 1) * D]
                )._wait_ge(*done).then_inc(s_sems[s], 16)

        # Head: the first compute op needs masks + cseg + pe tile 0.  Those
        # load as THREE parallel-ish DMAs: [masks|cseg] then pe0-low-half on
        # the SP ring, pe0-high-half on the ACT ring — halving the largest
        # serial transfer in front of the first op.  pe1 (only needed from
        # i=J/2) follows on SP.
        PE0 = PE_OFF
        PE0M = PE_OFF + D // 2
        @block.sync
        def _(sync):
            if dev_bcast:
                sync.dma_start(cw[0:1, :], cvec_d[:]).then_inc(cv_sem, 16)
            sync.dma_start(c_t[:, :PE0], const_d[:, :PE0]).then_inc(a_sem, 16)
            sync.dma_start(c_t[:, PE0:PE0M], const_d[:, PE0:PE0M]).then_inc(a_sem, 16)
            sync.dma_start(c_t[:, A_COLS:], const_d[:, A_COLS:]).then_inc(b_sem, 16)
            issue_out_dmas(sync, 0)

        @block.scalar
        def _(scalar):
            scalar.dma_start(
                c_t[:, PE0M:A_COLS], const_d[:, PE0M:A_COLS]
            ).then_inc(a_sem, 16)
            issue_out_dmas(scalar, 1)

        if dev_bcast:
            @block.gpsimd
            def _(gpsimd):
                from concourse import library_config
                # PartitionBroadcast lives in the attnmlp/attn/mlp/proxy
                # gpsimd ucode libraries, not the default one
                nc.gpsimd.load_library(library_config.attnmlp)
                gpsimd.wait_ge(cv_sem, 16)
                for v in range(NV):
                    nc.gpsimd.partition_broadcast(
                        cw[:, v * D:(v + 1) * D], cw[0:1, v * D:(v + 1) * D]
                    ).then_inc(bc_sem, 1)

        def compute(eng_handle, eng_obj, my_flag, done_sem):
            if dev_bcast:
                cseg = cw[:, 0:D]
            else:
                cseg = c_t[:, CSEG_OFF:CSEG_OFF + D]
            n_tok = 0
            waited_b = False
            eng_handle.wait_ge(a_sem, 48)   # all three pieces of the A-load
            if dev_bcast:
                eng_handle.wait_ge(bc_sem, NV)
            for i in range(J):
                if on_pool[i] != my_flag:
                    continue
                j = _order(i)
                t = j % T_TILES
                s = i % NSLOT
                if t == 1 and not waited_b:
                    eng_handle.wait_ge(b_sem, 16)
                    waited_b = True
                # slot-reuse wait attached inline on the STT (one wait max
                # per instruction; saves a sequencer dispatch slot)
                slot_wait = (
                    (s_sems[s], 16 * (i // NSLOT)) if i >= NSLOT else None
                )
                o_sl = obuf[:, s * D:(s + 1) * D]
                pe_sl = c_t[:, PE_OFF + t * D:PE_OFF + (t + 1) * D]
                m2_col = c_t[:, M2_OFF + j:M2_OFF + j + 1]
                if j in tok_cols:
                    m1_col = c_t[:, M1_OFF + j:M1_OFF + j + 1]
                    if dev_bcast:
                        ctok = cw[:, D:2 * D]
                    else:
                        ctok = c_t[:, CTOK_OFF:CTOK_OFF + D]
                    # acc = m1*c_tok + pe, then m2*c_seg + acc: the same fp
                    # add order as the reference's (tok + pe) + seg.
                    eng_obj.scalar_tensor_tensor(
                        o_sl, ctok, m1_col, pe_sl, op0=mult, op1=add,
                    )._maybe_wait_ge(slot_wait).then_inc(t_sem, 1)
                    n_tok += 1
                    eng_obj.scalar_tensor_tensor(
                        o_sl, cseg, m2_col, o_sl, op0=mult, op1=add,
                    )._wait_ge(t_sem, n_tok).then_inc(done_sem, 1)
                else:
                    eng_obj.scalar_tensor_tensor(
                        o_sl, cseg, m2_col, pe_sl, op0=mult, op1=add,
                    )._maybe_wait_ge(slot_wait).then_inc(done_sem, 1)

        @block.vector
        def _(vector):
            compute(vector, nc.vector, False, v_sem)

        if pool_split:
            @block.gpsimd
            def _(gpsimd):
                compute(gpsimd, nc.gpsimd, True, p_sem)

    nc.finalize()
    return nc


def _prepare_raw(inputs: dict, dev_bcast: bool = False):
    ids = np.asarray(inputs["input_ids"])
    seg = np.asarray(inputs["segment_label"])
    W_tok = np.asarray(inputs["W_tok"], dtype=np.float32)
    b_tok = np.asarray(inputs["b_tok"], dtype=np.float32)
    W_seg = np.asarray(inputs["W_seg"], dtype=np.float32)
    b_seg = np.asarray(inputs["b_seg"], dtype=np.float32)
    pe = np.asarray(inputs["pe"], dtype=np.float32).reshape(SEQ, D)

    c_tok = (W_tok[:, 0] + b_tok).astype(np.float32)
    c_seg = (W_seg[:, 0] + b_seg).astype(np.float32)
    m1_full = (ids == 0).astype(np.float32)
    m2_full = (seg == 0).astype(np.float32)

    per_core = []
    tok_cols = set()
    for c in range(N_CORES):
        sl = slice(c * S_SH, (c + 1) * S_SH)
        # [B, S_SH] -> [P, J] with column j = b*T_TILES + t, partition p
        m1 = m1_full[:, sl].reshape(B, T_TILES, P).transpose(2, 0, 1).reshape(P, J)
        m2 = m2_full[:, sl].reshape(B, T_TILES, P).transpose(2, 0, 1).reshape(P, J)
        pe_sl = pe[sl].reshape(T_TILES, P, D).transpose(1, 0, 2).reshape(P, T_TILES * D)
        tok_cols.update(np.nonzero(m1.any(axis=0))[0].tolist())
        per_core.append((pe_sl, m1, m2))

    tok_cols = frozenset(tok_cols)
    CSEG_OFF, M2_OFF, CTOK_OFF, M1_OFF, PE_OFF, C, _ = _layout(tok_cols, dev_bcast)
    NV = 2 if tok_cols else 1
    cvec = None
    if dev_bcast:
        cvec = np.empty((1, NV * D), dtype=np.float32)
        cvec[0, :D] = c_seg
        if tok_cols:
            cvec[0, D:] = c_tok
    in_maps = []
    for pe_sl, m1, m2 in per_core:
        consts = np.empty((P, C), dtype=np.float32)
        consts[:, M2_OFF:M2_OFF + J] = m2
        if tok_cols:
            consts[:, M1_OFF:M1_OFF + J] = m1
        if not dev_bcast:
            consts[:, CSEG_OFF:CSEG_OFF + D] = c_seg
            if tok_cols:
                consts[:, CTOK_OFF:CTOK_OFF + D] = c_tok
        consts[:, PE_OFF:PE_OFF + T_TILES * D] = pe_sl
        m = {"consts": consts}
        if dev_bcast:
            m["cvec"] = cvec
        in_maps.append(m)
    return in_maps, tok_cols


# -------------------------------------------------- tile variant (fallback) —

TPE_OFF = 0
TCSEG_OFF = T_TILES * D
TM2_OFF = TCSEG_OFF + D
TC_COMMON = TM2_OFF + J
TCTOK_OFF = TC_COMMON
TM1_OFF = TCTOK_OFF + D
TC_FULL = TM1_OFF + J


def _build_tile(tok_cols: frozenset) -> "bass.Bass":
    C = TC_FULL if tok_cols else TC_COMMON
    # Bacc (not plain Bass): its compile() splits multi-semaphore waits into
    # event semaphores — TRN2 allows at most one inline wait per instruction.
    nc = bacc.Bacc("TRN2", target_bir_lowering=False)
    const_d = nc.dram_tensor("consts", [P, C], _F32, kind="ExternalInput")
    out_d = nc.dram_tensor("out", [B * S_SH, D], _F32, kind="ExternalOutput")
    mult, add = mybir.AluOpType.mult, mybir.AluOpType.add

    with tile.TileContext(nc) as tc:
        with (
            tc.tile_pool(name="const", bufs=1) as cpool,
            tc.tile_pool(name="outp", bufs=8) as opool,
        ):
            c_t = cpool.tile([P, C], _F32)
            nc.sync.dma_start(c_t[:], const_d[:])
            cseg_t = c_t[:, TCSEG_OFF:TCSEG_OFF + D]
            ctok_t = c_t[:, TCTOK_OFF:TCTOK_OFF + D] if tok_cols else None

            for b in range(B):
                for t in range(T_TILES):
                    j = b * T_TILES + t
                    o = opool.tile([P, D], _F32, tag="out")
                    pe_slice = c_t[:, TPE_OFF + t * D:TPE_OFF + (t + 1) * D]
                    m2_col = c_t[:, TM2_OFF + j:TM2_OFF + j + 1]
                    if j in tok_cols:
                        m1_col = c_t[:, TM1_OFF + j:TM1_OFF + j + 1]
                        nc.vector.scalar_tensor_tensor(
                            o[:], ctok_t, m1_col, pe_slice, op0=mult, op1=add,
                        )
                        nc.vector.scalar_tensor_tensor(
                            o[:], cseg_t, m2_col, o[:], op0=mult, op1=add,
                        )
                    else:
                        nc.vector.scalar_tensor_tensor(
                            o[:], cseg_t, m2_col, pe_slice, op0=mult, op1=add,
                        )
                    nc.sync.dma_start(out_d[j * P:(j + 1) * P, :], o[:])
    nc.finalize()
    return nc


def _prepare_tile(inputs: dict):
    ids = np.asarray(inputs["input_ids"])
    seg = np.asarray(inputs["segment_label"])
    W_tok = np.asarray(inputs["W_tok"], dtype=np.float32)
    b_tok = np.asarray(inputs["b_tok"], dtype=np.float32)
    W_seg = np.asarray(inputs["W_seg"], dtype=np.float32)
    b_seg = np.asarray(inputs["b_seg"], dtype=np.float32)
    pe = np.asarray(inputs["pe"], dtype=np.float32).reshape(SEQ, D)

    c_tok = (W_tok[:, 0] + b_tok).astype(np.float32)
    c_seg = (W_seg[:, 0] + b_seg).astype(np.float32)
    m1_full = (ids == 0).astype(np.float32)
    m2_full = (seg == 0).astype(np.float32)

    per_core = []
    tok_cols = set()
    for c in range(N_CORES):
        sl = slice(c * S_SH, (c + 1) * S_SH)
        m1 = m1_full[:, sl].reshape(B, T_TILES, P).transpose(2, 0, 1).reshape(P, J)
        m2 = m2_full[:, sl].reshape(B, T_TILES, P).transpose(2, 0, 1).reshape(P, J)
        pe_sl = pe[sl].reshape(T_TILES, P, D).transpose(1, 0, 2).reshape(P, T_TILES * D)
        tok_cols.update(np.nonzero(m1.any(axis=0))[0].tolist())
        per_core.append((pe_sl, m1, m2))

    need_tok = bool(tok_cols)
    C = TC_FULL if need_tok else TC_COMMON
    in_maps = []
    for pe_sl, m1, m2 in per_core:
        consts = np.empty((P, C), dtype=np.float32)
        consts[:, TPE_OFF:TPE_OFF + T_TILES * D] = pe_sl
        consts[:, TCSEG_OFF:TCSEG_OFF + D] = c_seg
        consts[:, TM2_OFF:TM2_OFF + J] = m2
        if need_tok:
            consts[:, TCTOK_OFF:TCTOK_OFF + D] = c_tok
            consts[:, TM1_OFF:TM1_OFF + J] = m1
        in_maps.append({"consts": consts})
    return in_maps, frozenset(tok_cols)


# ------------------------------------------------------------------- entry —

def kernel(**inputs) -> np.ndarray:
    global LAST_RESULTS
    impl = os.environ.get("BASS_KERNEL_IMPL", "raw")
    if impl == "raw":
        # NOTE: pool_split compiles in CoreSim but walrus rejects
        # TensorScalarPtr on the Pool engine (NCC_IXCG966) — keep off.
        pool_split = bool(int(os.environ.get("BASS_KERNEL_POOL_SPLIT", "0")))
        dev_bcast = bool(int(os.environ.get("BASS_KERNEL_DEV_BCAST", "0")))
        in_maps, tok_cols = _prepare_raw(inputs, dev_bcast=dev_bcast)
        key = ("raw", pool_split, dev_bcast, tok_cols)
        def builder(tc):
            return _build_raw(tc, pool_split=pool_split, dev_bcast=dev_bcast)
    else:
        in_maps, tok_cols = _prepare_tile(inputs)
        key = ("tile", tok_cols)
        builder = _build_tile
    # SPMD: one program for all cores; the tok op is emitted for any column
    # that needs it on any core (a zero mask column makes it the identity).
    if key not in _prog_cache:
        _prog_cache[key] = builder(tok_cols)
    nc = _prog_cache[key]

    trace = bool(int(os.environ.get("BASS_KERNEL_TRACE", "0")))
    try:
        res = run_bass_kernel_spmd(
            nc, in_maps, list(range(N_CORES)), trace=trace,
            trace_cores=list(range(N_CORES)) if trace else None,
        )
    except ModuleNotFoundError:
        # axon builds without the NTFF profile hook (antenv.axon_hooks)
        # crash when tracing is requested (e.g. BASS_TRACE=1 in the env);
        # degrade to an untraced run rather than failing the kernel.
        os.environ["BASS_NEVER_TRACE"] = "1"
        res = run_bass_kernel_spmd(nc, in_maps, list(range(N_CORES)), trace=False)
    LAST_RESULTS = res

    out = np.empty((B, SEQ, D), dtype=np.float32)
    for c in range(N_CORES):
        out[:, c * S_SH:(c + 1) * S_SH, :] = (
            np.asarray(res.results[c]["out"]).reshape(B, S_SH, D)
        )
    return out



# revision 3
# speedup vs baseline: 1.7314x; 1.7314x over previous
"""Trainium2 Bass kernel for nn_BeBertEmbedding (self-contained).

Math: the reference's semantic_embed(ids, W, b, pad=0) is
    where(ids==0, take(W.T, ids) + b, zeros)
so the only table row that survives is W[:, 0], and the whole module is
    out[b,s,:] = pe[s,:] + (ids[b,s]==0)*(W_tok[:,0]+b_tok)
                         + (seg[b,s]==0)*(W_seg[:,0]+b_seg)

Sharding: sequence-parallel across 8 cores (256 positions/core, all 16
batches; each core writes a disjoint [16, 256, 768] slice, no collectives).

v2 (this file): fp16 datapath end-to-end (the rel-err gate is 2e-2;
fp16 rounding is ~5e-4), which halves both the HBM write traffic and the
modeled DMA cost, plus a three-stream compute schedule so no single
engine carries all 32 output tiles:

  DVE : per tile  corr = cseg*m2[p]   (TensorScalar, 4x DVE perf mode)
        then      out  = corr + pe    (TensorTensor, 2x mode; emitted
        pairwise over two adjacent slots with a stride-0-broadcast pe to
        share the op init)
  PE  : per tile 4 matmuls — [m2col;m1col]@[cseg;ctok] + I@pe_t — into
        fp32 PSUM (two 512/256 col slices per 2KB-bank constraint); tok
        (ids==0) tiles are forced onto this stream so the m1 row rides
        along for free
  ACT : paired activation-Copy PSUM->SBUF fp16 over two psum banks
  SP / ACT / Pool(SWDGE) queues: inputs + out-DMAs, tails routed to
        whichever queue drains first (Pool's non-pipelined ~1.9us SWDGE
        latency keeps it off the critical tail)

Cost-model: 19.2us/core (baseline f32 single-DVE-stream: 33.2us).
DVE stream ~14.5us is the critical path; total = DVE end + ~2.8us of
fixed drain (last out-DMA + 900ns DMA-sem prop + end-of-block barrier).
"""

import contextlib
import os
import sys

import numpy as np

try:
    from concourse import bass, mybir
    from concourse.bass_utils import run_bass_kernel_spmd
except ImportError:
    for _p in ("/opt/trn_rl_repo", "/root/.axon_site/_ro/trn_rl_repo"):
        if os.path.isdir(_p) and _p not in sys.path:
            sys.path.insert(0, _p)
            break
    from concourse import bass, mybir
    from concourse.bass_utils import run_bass_kernel_spmd

N_CORES = 8
B, SEQ, D = 16, 2048, 768
S_SH = SEQ // N_CORES        # 256 sequence positions per core
P = 128                      # partitions
T_TILES = S_SH // P          # 2 seq tiles per core
J = B * T_TILES              # 32 output tiles per core

FP16 = mybir.dt.float16
F32 = mybir.dt.float32
ACopy = mybir.ActivationFunctionType.Copy
ADD = mybir.AluOpType.add

# ---- layout of c16 [128, C16] fp16 ----
CSEG_O = 0
PE0_O = CSEG_O + D
PE1_O = PE0_O + D
IDM_O = PE1_O + D
C16 = IDM_O + P

PSW = 1024                   # psum cols per tile (2 banks, 768 used)

N_PE_DEFAULT = 12            # tiles on the PE+ACT stream (even)
NS_D = 12                    # reused DVE slots
NS_A = 8                     # reused ACT slots (even)
LA = 4                       # DVE ts_mul lookahead

_prog_cache: dict = {}
LAST_RESULTS = None


def _order_tiles(pe_tiles):
    """t=0 tiles first within each stream; returns (dve_list, pe_list)."""
    pe_set = set(pe_tiles)
    t0 = [j for j in range(J) if j % T_TILES == 0]
    t1 = [j for j in range(J) if j % T_TILES == 1]
    ordered = t0 + t1
    return ([j for j in ordered if j not in pe_set],
            [j for j in ordered if j in pe_set])


def _build(pe_tiles, ns_d=NS_D, ns_a=NS_A, la=LA):
    dve_tiles, pe_list = _order_tiles(pe_tiles)
    ND = len(dve_tiles)
    N_PE = len(pe_list)
    n_pairs = (N_PE + 1) // 2
    FRESH_D = min(4, ND)
    NR = ND - FRESH_D
    NS = ns_d + ns_a + FRESH_D + 2
    n_head = min(4, max(N_PE, 1))

    nc = bass.Bass("TRN2")
    c16_d = nc.dram_tensor("c16", [P, C16], FP16, kind="ExternalInput")
    cm_d = nc.dram_tensor("cm", [P, J], F32, kind="ExternalInput")
    c2v_d = nc.dram_tensor("c2v", [2, D], FP16, kind="ExternalInput")
    c2mh_d = nc.dram_tensor("c2mh", [2, n_head * P], FP16, kind="ExternalInput")
    c2mr_d = (nc.dram_tensor("c2mr", [2, (N_PE - n_head) * P], FP16,
                             kind="ExternalInput") if N_PE > n_head else None)
    out_d = nc.dram_tensor("out", [B * S_SH, D], FP16, kind="ExternalOutput")

    def dslot(i):
        return i % ns_d if i < NR else ns_d + ns_a + (i - NR)

    def dve_on_pool(i):
        if i < NR:
            return (i % ns_d) % 3 == 2
        return (i - NR) % 2 == 1

    def aslot(k):
        q = k // 2
        if q == n_pairs - 1:
            return ns_d + ns_a + FRESH_D + (k % 2)
        return ns_d + (k % ns_a)

    with contextlib.ExitStack() as stack:
        c16 = stack.enter_context(nc.sbuf_tensor("c16s", [P, C16], FP16))
        cm = stack.enter_context(nc.sbuf_tensor("cms", [P, J], F32))
        c2v = stack.enter_context(nc.sbuf_tensor("c2vs", [2, D], FP16))
        c2m = stack.enter_context(
            nc.sbuf_tensor("c2ms", [2, max(N_PE, 1) * P], FP16))
        NH = n_head * P
        obuf = stack.enter_context(nc.sbuf_tensor("obuf", [P, NS * D], FP16))
        psa = stack.enter_context(nc.psum_tensor("psa", [P, 4 * PSW], F32))
        warm = stack.enter_context(nc.sbuf_tensor("warm", [P, 512], FP16))
        s_a = stack.enter_context(nc.semaphore("s_a"))
        s_p = stack.enter_context(nc.semaphore("s_p"))
        s_b = stack.enter_context(nc.semaphore("s_b"))
        s_c = stack.enter_context(nc.semaphore("s_c"))
        s_m = stack.enter_context(nc.semaphore("s_m"))      # SWDGE-only
        s_mh = stack.enter_context(nc.semaphore("s_mh"))
        s_fin = stack.enter_context(nc.semaphore("s_fin"))  # HWDGE fresh-slot
        s_finp = stack.enter_context(nc.semaphore("s_finp"))  # SWDGE fresh-slot
        w_sem = stack.enter_context(nc.semaphore("w_sem"))
        v_sem = stack.enter_context(nc.semaphore("v_sem"))
        t_sem = stack.enter_context(nc.semaphore("t_sem"))
        p_sem = stack.enter_context(nc.semaphore("p_sem"))
        c_sem = stack.enter_context(nc.semaphore("c_sem"))
        sd = [stack.enter_context(nc.semaphore(f"sd{i}")) for i in range(ns_d)]
        sa = [stack.enter_context(nc.semaphore(f"sa{i}")) for i in range(ns_a)]
        block = stack.enter_context(nc.Block())

        cseg = c16[:, CSEG_O:CSEG_O + D]
        idm = c16[:, IDM_O:IDM_O + P]

        def pe_sl(j):
            o = PE0_O if j % T_TILES == 0 else PE1_O
            return c16[:, o:o + D]

        # DVE tt pairing (adjacent slots, same t, reused region only)
        paired = [False] * ND
        for i in range(0, NR - 1, 2):
            if dve_tiles[i] % T_TILES == dve_tiles[i + 1] % T_TILES:
                paired[i] = True
        vneed = [0] * ND
        n_tt = 0
        i = 0
        while i < ND:
            n_tt += 1
            if paired[i]:
                vneed[i] = vneed[i + 1] = n_tt
                i += 2
            else:
                vneed[i] = n_tt
                i += 1

        # ---------------- SP: inputs + SP out-DMAs ----------------
        @block.sync
        def _(sync):
            sync.dma_start(c16[:, CSEG_O:CSEG_O + D],
                           c16_d[:, CSEG_O:CSEG_O + D]).then_inc(s_a, 16)
            sync.dma_start(cm[:], cm_d[:]).then_inc(s_a, 16)
            sync.dma_start(c16[:, PE1_O:PE1_O + D],
                           c16_d[:, PE1_O:PE1_O + D]).then_inc(s_b, 16)
            for i, j in enumerate(dve_tiles):
                if dve_on_pool(i):
                    continue
                s = dslot(i)
                fin = sd[s] if i < NR else s_fin
                sync.dma_start(out_d[j * P:(j + 1) * P, :],
                               obuf[:, s * D:(s + 1) * D]
                               )._wait_ge(v_sem, vneed[i]).then_inc(fin, 16)
            if N_PE >= 1:
                k = 2 * (n_pairs - 1)
                j = pe_list[k]
                s = aslot(k)
                sync.dma_start(out_d[j * P:(j + 1) * P, :],
                               obuf[:, s * D:(s + 1) * D]
                               )._wait_ge(c_sem, n_pairs).then_inc(s_fin, 16)

        # ---------------- ACT: inputs + paired activations ----------------
        @block.scalar
        def _(scalar):
            scalar.dma_start(c16[:, PE0_O:PE0_O + D],
                             c16_d[:, PE0_O:PE0_O + D]).then_inc(s_p, 16)
            scalar.dma_start(c2m[:, 0:NH], c2mh_d[:]).then_inc(s_mh, 16)
            scalar.dma_start(c2v[:], c2v_d[:]).then_inc(s_c, 16)
            scalar.dma_start(c16[:, IDM_O:IDM_O + P],
                             c16_d[:, IDM_O:IDM_O + P]).then_inc(s_c, 16)
            for q in range(n_pairs):
                k0 = 2 * q
                npair = min(2, N_PE - k0)
                s = aslot(k0)
                if q < n_pairs - 1:
                    for kk in range(npair):
                        if k0 + kk >= ns_a:
                            scalar.wait_ge(sa[(k0 + kk) % ns_a],
                                           16 * ((k0 + kk) // ns_a))
                bank = (k0 % 4) * PSW
                if npair == 2 and aslot(k0 + 1) == s + 1:
                    src = psa[:, bank:bank + 2 * PSW].rearrange(
                        "p (b c) -> p b c", b=2)[:, :, 0:D]
                    dst = obuf[:, s * D:(s + 2) * D].rearrange(
                        "p (b c) -> p b c", b=2)
                    nc.scalar.activation(dst, src, ACopy,
                                         )._wait_ge(p_sem, q + 1).then_inc(c_sem, 1)
                else:
                    for kk in range(npair):
                        act = nc.scalar.activation(
                            obuf[:, (s + kk) * D:(s + kk + 1) * D],
                            psa[:, bank + kk * PSW:bank + kk * PSW + D], ACopy,
                        )._wait_ge(p_sem, q + 1)
                        if kk == npair - 1:
                            act.then_inc(c_sem, 1)
            # drain the second tile of the last pair on the now-idle ACT ring
            if N_PE >= 2:
                k = 2 * (n_pairs - 1) + 1
                if k < N_PE:
                    j = pe_list[k]
                    s = aslot(k)
                    scalar.dma_start(out_d[j * P:(j + 1) * P, :],
                                     obuf[:, s * D:(s + 1) * D]
                                     )._wait_ge(c_sem, n_pairs).then_inc(s_fin, 16)

        # ---------------- DVE stream ----------------
        @block.vector
        def _(vector):
            nc.vector.memset(warm[:], 0.0).then_inc(w_sem, 1)
            vector.wait_ge(s_a, 32)
            waited_p = waited_b = False

            def emit_ts(i):
                s = dslot(i)
                ts = nc.vector.tensor_scalar_mul(
                    obuf[:, s * D:(s + 1) * D], cseg,
                    cm[:, dve_tiles[i]:dve_tiles[i] + 1])
                if i < NR and i >= ns_d:
                    ts._wait_ge(sd[s], 16 * (i // ns_d))
                ts.then_inc(t_sem, 1)

            def wait_pe(t):
                nonlocal waited_p, waited_b
                if t == 0 and not waited_p:
                    vector.wait_ge(s_p, 16)
                    waited_p = True
                if t == 1 and not waited_b:
                    vector.wait_ge(s_b, 16)
                    waited_b = True

            for i in range(min(la, ND)):
                emit_ts(i)
            i = 0
            while i < ND:
                j = dve_tiles[i]
                wait_pe(j % T_TILES)
                s = dslot(i)
                if paired[i]:
                    pe_b = pe_sl(j).unsqueeze(1).broadcast_to([P, 2, D])
                    dst = obuf[:, s * D:(s + 2) * D].rearrange(
                        "p (b c) -> p b c", b=2)
                    nc.vector.tensor_tensor(
                        dst, dst, pe_b, op=ADD,
                    )._wait_ge(t_sem, i + 2).then_inc(v_sem, 1)
                    adv = 2
                else:
                    o_sl = obuf[:, s * D:(s + 1) * D]
                    nc.vector.tensor_tensor(
                        o_sl, o_sl, pe_sl(j), op=ADD,
                    )._wait_ge(t_sem, i + 1).then_inc(v_sem, 1)
                    adv = 1
                for z in range(adv):
                    if i + la + z < ND:
                        emit_ts(i + la + z)
                i += adv

        # ---------------- PE stream ----------------
        @block.tensor
        def _(tensor):
            tensor.wait_ge(w_sem, 1)
            for _w in range(5):
                nc.tensor.matmul(psa[:, 0:512], warm[:, 0:P], warm[:],
                                 start=True, stop=True)
            tensor.wait_ge(s_c, 32)
            tensor.wait_ge(s_mh, 16)
            waited_p = waited_b = False
            for k, j in enumerate(pe_list):
                t = j % T_TILES
                bank = (k % 4) * PSW
                q = k // 2
                lhsT = c2m[0:2, k * P:(k + 1) * P]
                if k == n_head and c2mr_d is not None:
                    tensor.wait_ge(s_m, 16)
                if t == 0 and not waited_p:
                    tensor.wait_ge(s_p, 16)
                    waited_p = True
                if t == 1 and not waited_b:
                    tensor.wait_ge(s_b, 16)
                    waited_b = True
                mm = nc.tensor.matmul(psa[:, bank:bank + 512], lhsT,
                                      c2v[:, 0:512], start=True, stop=False)
                if k >= 4:
                    mm._wait_ge(c_sem, q - 1)
                nc.tensor.matmul(psa[:, bank + 512:bank + D], lhsT,
                                 c2v[:, 512:D], start=True, stop=False)
                nc.tensor.matmul(psa[:, bank:bank + 512], idm,
                                 pe_sl(j)[:, 0:512], start=False, stop=True)
                mm4 = nc.tensor.matmul(psa[:, bank + 512:bank + D], idm,
                                       pe_sl(j)[:, 512:D], start=False, stop=True)
                if k % 2 == 1 or k == N_PE - 1:
                    mm4.then_inc(p_sem, 1)

        # ---------------- Pool: c2m rest + out-DMAs ----------------
        @block.gpsimd
        def _(gpsimd):
            if c2mr_d is not None:
                gpsimd.dma_start(c2m[:, NH:], c2mr_d[:]).then_inc(s_m, 16)
            work = []
            for k, j in enumerate(pe_list):
                q = k // 2
                if q == n_pairs - 1:
                    continue
                work.append((3600 + (q + 1) * 1480, "a", k, j))
            for i, j in enumerate(dve_tiles):
                if dve_on_pool(i):
                    work.append((2400 + (i + 1) * 720, "d", i, j))
            for est, kind, idx, j in sorted(work):
                if kind == "a":
                    s = aslot(idx)
                    gpsimd.dma_start(out_d[j * P:(j + 1) * P, :],
                                     obuf[:, s * D:(s + 1) * D]
                                     )._wait_ge(c_sem, idx // 2 + 1
                                                ).then_inc(sa[idx % ns_a], 16)
                else:
                    s = dslot(idx)
                    fin = sd[s] if idx < NR else s_finp
                    gpsimd.dma_start(out_d[j * P:(j + 1) * P, :],
                                     obuf[:, s * D:(s + 1) * D]
                                     )._wait_ge(v_sem, vneed[idx]).then_inc(fin, 16)

    nc.finalize()
    return nc, dve_tiles, pe_list


def _choose_pe_tiles(tok_cols):
    """Even-sized PE-stream tile set containing every tok column."""
    pe = sorted(tok_cols)
    for j in range(0, J, 2):          # prefer t=0 tiles (j even)
        if len(pe) >= N_PE_DEFAULT and len(pe) % 2 == 0:
            break
        if j not in tok_cols:
            pe.append(j)
    if len(pe) % 2:                   # pad to even with any spare tile
        for j in range(J):
            if j not in pe:
                pe.append(j)
                break
    return tuple(sorted(pe))


def _prepare(inputs):
    ids = np.asarray(inputs["input_ids"])
    seg = np.asarray(inputs["segment_label"])
    W_tok = np.asarray(inputs["W_tok"], dtype=np.float32)
    b_tok = np.asarray(inputs["b_tok"], dtype=np.float32)
    W_seg = np.asarray(inputs["W_seg"], dtype=np.float32)
    b_seg = np.asarray(inputs["b_seg"], dtype=np.float32)
    pe = np.asarray(inputs["pe"], dtype=np.float32).reshape(SEQ, D)

    c_tok = (W_tok[:, 0] + b_tok).astype(np.float32)
    c_seg = (W_seg[:, 0] + b_seg).astype(np.float32)
    m1_full = (ids == 0).astype(np.float32)
    m2_full = (seg == 0).astype(np.float32)

    per_core = []
    tok_cols = set()
    for c in range(N_CORES):
        sl = slice(c * S_SH, (c + 1) * S_SH)
        # [B, S_SH] -> [P, J]: column j = b*T_TILES + t, partition p
        m1 = m1_full[:, sl].reshape(B, T_TILES, P).transpose(2, 0, 1).reshape(P, J)
        m2 = m2_full[:, sl].reshape(B, T_TILES, P).transpose(2, 0, 1).reshape(P, J)
        pe_sl = pe[sl].reshape(T_TILES, P, D)
        tok_cols.update(np.nonzero(m1.any(axis=0))[0].tolist())
        per_core.append((pe_sl, m1, m2))

    pe_tiles = _choose_pe_tiles(tok_cols)
    _, pe_list = _order_tiles(pe_tiles)
    n_pe = len(pe_list)
    n_head = min(4, max(n_pe, 1))

    in_maps = []
    for pe_sl, m1, m2 in per_core:
        c16 = np.zeros((P, C16), np.float16)
        c16[:, CSEG_O:CSEG_O + D] = c_seg[None, :].astype(np.float16)
        c16[:, PE0_O:PE0_O + D] = pe_sl[0].astype(np.float16)
        c16[:, PE1_O:PE1_O + D] = pe_sl[1].astype(np.float16)
        c16[:, IDM_O:IDM_O + P] = np.eye(P, dtype=np.float16)
        cmv = m2.astype(np.float32)
        c2v = np.zeros((2, D), np.float16)
        c2v[0] = c_seg.astype(np.float16)
        c2v[1] = c_tok.astype(np.float16)
        c2m = np.zeros((2, max(n_pe, 1) * P), np.float16)
        for k, j in enumerate(pe_list):
            c2m[0, k * P:(k + 1) * P] = m2[:, j].astype(np.float16)
            c2m[1, k * P:(k + 1) * P] = m1[:, j].astype(np.float16)
        m = {"c16": c16, "cm": cmv, "c2v": c2v, "c2mh": c2m[:, :n_head * P]}
        if n_pe > n_head:
            m["c2mr"] = c2m[:, n_head * P:]
        in_maps.append(m)
    return in_maps, pe_tiles


def kernel(**inputs) -> np.ndarray:
    global LAST_RESULTS
    in_maps, pe_tiles = _prepare(inputs)
    key = ("v2", pe_tiles)
    if key not in _prog_cache:
        _prog_cache[key] = _build(pe_tiles)[0]
    nc = _prog_cache[key]

    trace = bool(int(os.environ.get("BASS_KERNEL_TRACE", "0")))
    try:
        res = run_bass_kernel_spmd(
            nc, in_maps, list(range(N_CORES)), trace=trace,
            trace_cores=list(range(N_CORES)) if trace else None,
        )
    except ModuleNotFoundError:
        # axon builds without the NTFF profile hook crash when tracing is
        # requested; degrade to an untraced run.
        os.environ["BASS_NEVER_TRACE"] = "1"
        res = run_bass_kernel_spmd(nc, in_maps, list(range(N_CORES)), trace=False)
    LAST_RESULTS = res

    out = np.empty((B, SEQ, D), dtype=np.float32)
    for c in range(N_CORES):
        out[:, c * S_SH:(c + 1) * S_SH, :] = (
            np.asarray(res.results[c]["out"])
            .astype(np.float32).reshape(B, S_SH, D)
        )
    return out


# revision 6
# speedup vs baseline: 1.7455x; 1.0081x over previous
"""Trainium2 Bass kernel for nn_BeBertEmbedding (self-contained).

Math: the reference's semantic_embed(ids, W, b, pad=0) is
    where(ids==0, take(W.T, ids) + b, zeros)
so the only table row that survives is W[:, 0], and the whole module is
    out[b,s,:] = pe[s,:] + (ids[b,s]==0)*(W_tok[:,0]+b_tok)
                         + (seg[b,s]==0)*(W_seg[:,0]+b_seg)

Sharding: sequence-parallel across 8 cores (256 positions/core, all 16
batches; each core writes a disjoint [16, 256, 768] slice, no collectives).

v2 (this file): fp16 datapath end-to-end (the rel-err gate is 2e-2;
fp16 rounding is ~5e-4), which halves both the HBM write traffic and the
modeled DMA cost, plus a three-stream compute schedule so no single
engine carries all 32 output tiles:

  DVE : per tile  corr = cseg*m2[p]   (TensorScalar, 4x DVE perf mode)
        then      out  = corr + pe    (TensorTensor, 2x mode; emitted
        pairwise over two adjacent slots with a stride-0-broadcast pe to
        share the op init)
  PE  : per tile 4 matmuls — [m2col;m1col]@[cseg;ctok] + I@pe_t — into
        fp32 PSUM (two 512/256 col slices per 2KB-bank constraint); tok
        (ids==0) tiles are forced onto this stream so the m1 row rides
        along for free
  ACT : paired activation-Copy PSUM->SBUF fp16 over two psum banks
  SP / ACT / Pool(SWDGE) queues: inputs + out-DMAs, tails routed to
        whichever queue drains first (Pool's non-pipelined ~1.9us SWDGE
        latency keeps it off the critical tail)

Cost-model: 19.2us/core (baseline f32 single-DVE-stream: 33.2us).
DVE stream ~14.5us is the critical path; total = DVE end + ~2.8us of
fixed drain (last out-DMA + 900ns DMA-sem prop + end-of-block barrier).
"""

import contextlib
import os
import sys

import numpy as np

try:
    from concourse import bass, mybir
    from concourse.bass_utils import run_bass_kernel_spmd
except ImportError:
    for _p in ("/opt/trn_rl_repo", "/root/.axon_site/_ro/trn_rl_repo"):
        if os.path.isdir(_p) and _p not in sys.path:
            sys.path.insert(0, _p)
            break
    from concourse import bass, mybir
    from concourse.bass_utils import run_bass_kernel_spmd

N_CORES = 8
B, SEQ, D = 16, 2048, 768
S_SH = SEQ // N_CORES        # 256 sequence positions per core
P = 128                      # partitions
T_TILES = S_SH // P          # 2 seq tiles per core
J = B * T_TILES              # 32 output tiles per core

FP16 = mybir.dt.float16
F32 = mybir.dt.float32
ACopy = mybir.ActivationFunctionType.Copy
ADD = mybir.AluOpType.add

# ---- layout of c16 [128, C16] fp16 ----
CSEG_O = 0
PE0_O = CSEG_O + D
PE1_O = PE0_O + D
IDM_O = PE1_O + D
C16 = IDM_O + P

PSW = 1024                   # psum cols per tile (2 banks, 768 used)

N_PE_DEFAULT = 12            # tiles on the PE+ACT stream (even)
NS_D = 12                    # reused DVE slots
NS_A = 8                     # reused ACT slots (even)
LA = 4                       # DVE ts_mul lookahead

_prog_cache: dict = {}
LAST_RESULTS = None


def _order_tiles(pe_tiles):
    """t=0 tiles first within each stream; returns (dve_list, pe_list)."""
    pe_set = set(pe_tiles)
    t0 = [j for j in range(J) if j % T_TILES == 0]
    t1 = [j for j in range(J) if j % T_TILES == 1]
    ordered = t0 + t1
    return ([j for j in ordered if j not in pe_set],
            [j for j in ordered if j in pe_set])


def _build(pe_tiles, ns_d=NS_D, ns_a=NS_A, la=LA):
    """pe_tiles: tile indices on the PE+ACT stream."""
    dve_tiles, pe_list = _order_tiles(pe_tiles)
    ND = len(dve_tiles)
    N_PE = len(pe_list)
    n_pairs = (N_PE + 1) // 2
    FRESH_D = min(4, ND)           # last DVE tiles on fresh slots
    NR = ND - FRESH_D              # reused-slot DVE tiles
    NS = ns_d + ns_a + FRESH_D + 2
    n_head = min(4, max(N_PE, 1))

    nc = bass.Bass("TRN2")
    c16_d = nc.dram_tensor("c16", [P, C16], FP16, kind="ExternalInput")
    cm_d = nc.dram_tensor("cm", [P, J], F32, kind="ExternalInput")
    c2v_d = nc.dram_tensor("c2v", [2, D], FP16, kind="ExternalInput")
    c2mh_d = nc.dram_tensor("c2mh", [2, n_head * P], FP16, kind="ExternalInput")
    c2mr_d = (nc.dram_tensor("c2mr", [2, (N_PE - n_head) * P], FP16,
                             kind="ExternalInput") if N_PE > n_head else None)
    out_d = nc.dram_tensor("out", [B * S_SH, D], FP16, kind="ExternalOutput")

    # slot assignment for DVE tiles
    def dslot(i):
        return i % ns_d if i < NR else ns_d + ns_a + (i - NR)

    def dve_on_pool(i):
        if i < NR:
            return (i % ns_d) % 3 == 2
        return False  # fresh tails stay off Pool (SWDGE tail latency)     # fresh tiles alternate SP/Pool

    # ACT pair q slots; last pair gets fresh slots
    def aslot(k):
        q = k // 2
        if q == n_pairs - 1:
            return ns_d + ns_a + FRESH_D + (k % 2)
        return ns_d + (k % ns_a)

    with contextlib.ExitStack() as stack:
        c16 = stack.enter_context(nc.sbuf_tensor("c16s", [P, C16], FP16))
        cm = stack.enter_context(nc.sbuf_tensor("cms", [P, J], F32))
        c2v = stack.enter_context(nc.sbuf_tensor("c2vs", [2, D], FP16))
        c2m = stack.enter_context(
            nc.sbuf_tensor("c2ms", [2, max(N_PE, 1) * P], FP16))
        NH = n_head * P
        obuf = stack.enter_context(nc.sbuf_tensor("obuf", [P, NS * D], FP16))
        psa = stack.enter_context(nc.psum_tensor("psa", [P, 4 * PSW], F32))
        warm = stack.enter_context(nc.sbuf_tensor("warm", [P, 512], FP16))
        s_a = stack.enter_context(nc.semaphore("s_a"))    # cseg + cm
        s_p = stack.enter_context(nc.semaphore("s_p"))    # pe0
        s_b = stack.enter_context(nc.semaphore("s_b"))    # pe1
        s_c = stack.enter_context(nc.semaphore("s_c"))    # c2v + idm
        s_m = stack.enter_context(nc.semaphore("s_m"))    # c2m rest (SWDGE-only)
        s_mh = stack.enter_context(nc.semaphore("s_mh"))  # c2m head
        s_fin = stack.enter_context(nc.semaphore("s_fin"))    # HWDGE fresh-slot
        s_finp = stack.enter_context(nc.semaphore("s_finp"))  # SWDGE fresh-slot
        w_sem = stack.enter_context(nc.semaphore("w_sem"))
        v_sem = stack.enter_context(nc.semaphore("v_sem"))
        t_sem = stack.enter_context(nc.semaphore("t_sem"))
        p_sem = stack.enter_context(nc.semaphore("p_sem"))
        c_sem = stack.enter_context(nc.semaphore("c_sem"))
        sd = [stack.enter_context(nc.semaphore(f"sd{i}")) for i in range(ns_d)]
        sa = [stack.enter_context(nc.semaphore(f"sa{i}")) for i in range(ns_a)]
        sh = [stack.enter_context(nc.semaphore(f"sh{i}")) for i in range(2)]
        block = stack.enter_context(nc.Block())

        cseg = c16[:, CSEG_O:CSEG_O + D]
        idm = c16[:, IDM_O:IDM_O + P]

        def pe_sl(j):
            o = PE0_O if j % T_TILES == 0 else PE1_O
            return c16[:, o:o + D]

        # DVE tt grouping: group of G consecutive tiles, same t, adjacent
        # slots, all in the reused region.  ns_d must be a multiple of 4.
        group = [1] * ND
        i = 0
        CANDS = lambda i: (2,)
        while i < NR:
            g = 1
            cands = CANDS(i)
            for cand in cands:
                if (i % cand == 0 and i + cand <= NR
                        and (i % ns_d) + cand <= ns_d
                        and len({dve_tiles[i + z] % T_TILES
                                 for z in range(cand)}) == 1):
                    g = cand
                    break
            group[i] = g
            i += g
        vneed = [0] * ND
        n_tt = 0
        i = 0
        while i < ND:
            n_tt += 1
            g = group[i] if i < NR else 1
            for z in range(g):
                vneed[i + z] = n_tt
            i += g

        emitted_act = [False]

        # ---------------- SP: inputs + SP out-DMAs ----------------
        @block.sync
        def _(sync):
            sync.dma_start(c16[:, CSEG_O:CSEG_O + D],
                           c16_d[:, CSEG_O:CSEG_O + D]).then_inc(s_a, 16)
            sync.dma_start(cm[:], cm_d[:]).then_inc(s_a, 16)
            sync.dma_start(c16[:, PE1_O:PE1_O + D],
                           c16_d[:, PE1_O:PE1_O + D]).then_inc(s_b, 16)
            sp_dve = [(i, j) for i, j in enumerate(dve_tiles)
                      if not dve_on_pool(i)]
            for i, j in sp_dve:
                if i >= NR and (i - NR) % 2 == 1:
                    continue  # drained on the ACT ring
                if (N_PE >= 2 and N_PE % 2 == 0 and i >= NR
                        and not emitted_act[0]):
                    # the last ACT pair's first tile is ready before the
                    # fresh DVE tails -- drain it first
                    k = 2 * (n_pairs - 1)
                    ja = pe_list[k]
                    sa_ = aslot(k)
                    sync.dma_start(out_d[ja * P:(ja + 1) * P, :],
                                   obuf[:, sa_ * D:(sa_ + 1) * D]
                                   )._wait_ge(c_sem, n_pairs).then_inc(s_fin, 16)
                    emitted_act[0] = True
                s = dslot(i)
                fin = sd[s] if i < NR else s_fin
                sync.dma_start(out_d[j * P:(j + 1) * P, :],
                               obuf[:, s * D:(s + 1) * D]
                               )._wait_ge(v_sem, vneed[i]).then_inc(fin, 16)
            if N_PE >= 2 and N_PE % 2 == 0 and not emitted_act[0]:
                k = 2 * (n_pairs - 1)
                ja = pe_list[k]
                sa_ = aslot(k)
                sync.dma_start(out_d[ja * P:(ja + 1) * P, :],
                               obuf[:, sa_ * D:(sa_ + 1) * D]
                               )._wait_ge(c_sem, n_pairs).then_inc(s_fin, 16)

        # ---------------- ACT: inputs + paired activations ----------------
        @block.scalar
        def _(scalar):
            scalar.dma_start(c16[:, PE0_O:PE0_O + D],
                             c16_d[:, PE0_O:PE0_O + D]).then_inc(s_p, 16)
            scalar.dma_start(c2m[:, 0:NH], c2mh_d[:]).then_inc(s_mh, 16)
            scalar.dma_start(c2v[:], c2v_d[:]).then_inc(s_c, 16)
            scalar.dma_start(c16[:, IDM_O:IDM_O + P],
                             c16_d[:, IDM_O:IDM_O + P]).then_inc(s_c, 16)
            for q in range(n_pairs):
                k0 = 2 * q
                npair = min(2, N_PE - k0)
                s = aslot(k0)
                if q < n_pairs - 1:
                    for kk in range(npair):
                        if k0 + kk >= ns_a:
                            sl = (k0 + kk) % ns_a
                            sem_r = sh[sl - 6] if sl >= 6 else sa[sl]
                            scalar.wait_ge(sem_r, 16 * ((k0 + kk) // ns_a))
                bank = (k0 % 4) * PSW
                if npair == 2 and aslot(k0 + 1) == s + 1:
                    src = psa[:, bank:bank + 2 * PSW].rearrange(
                        "p (b c) -> p b c", b=2)[:, :, 0:D]
                    dst = obuf[:, s * D:(s + 2) * D].rearrange(
                        "p (b c) -> p b c", b=2)
                    nc.scalar.activation(dst, src, ACopy,
                                         )._wait_ge(p_sem, q + 1).then_inc(c_sem, 1)
                else:
                    for kk in range(npair):
                        act = nc.scalar.activation(
                            obuf[:, (s + kk) * D:(s + kk + 1) * D],
                            psa[:, bank + kk * PSW:bank + kk * PSW + D], ACopy,
                        )._wait_ge(p_sem, q + 1)
                        if kk == npair - 1:
                            act.then_inc(c_sem, 1)
            if N_PE >= 1:
                k = 2 * (n_pairs - 1) + 1 if N_PE % 2 == 0 else N_PE - 1
                if k < N_PE:
                    j = pe_list[k]
                    s = aslot(k)
                    scalar.dma_start(out_d[j * P:(j + 1) * P, :],
                                     obuf[:, s * D:(s + 1) * D]
                                     )._wait_ge(c_sem, n_pairs).then_inc(s_fin, 16)
            for i, j in enumerate(dve_tiles):
                if i >= NR and (i - NR) % 2 == 1:
                    s = dslot(i)
                    scalar.dma_start(out_d[j * P:(j + 1) * P, :],
                                     obuf[:, s * D:(s + 1) * D]
                                     )._wait_ge(v_sem, vneed[i]).then_inc(s_fin, 16)

        # ---------------- DVE stream ----------------
        @block.vector
        def _(vector):
            nc.vector.memset(warm[:], 0.0).then_inc(w_sem, 1)
            vector.wait_ge(s_a, 32)
            waited_p = waited_b = False

            def emit_ts(i):
                s = dslot(i)
                ts = nc.vector.tensor_scalar_mul(
                    obuf[:, s * D:(s + 1) * D], cseg, cm[:, dve_tiles[i]:dve_tiles[i] + 1])
                if i < NR and i >= ns_d:
                    ts._wait_ge(sd[s], 16 * (i // ns_d))
                ts.then_inc(t_sem, 1)

            def wait_pe(t):
                nonlocal waited_p, waited_b
                if t == 0 and not waited_p:
                    vector.wait_ge(s_p, 16)
                    waited_p = True
                if t == 1 and not waited_b:
                    vector.wait_ge(s_b, 16)
                    waited_b = True

            for i in range(min(la, ND)):
                emit_ts(i)
            i = 0
            while i < ND:
                j = dve_tiles[i]
                wait_pe(j % T_TILES)
                s = dslot(i)
                g = group[i] if i < NR else 1
                if g > 1:
                    pe_b = pe_sl(j).unsqueeze(1).broadcast_to([P, g, D])
                    dst = obuf[:, s * D:(s + g) * D].rearrange(
                        "p (b c) -> p b c", b=g)
                    nc.vector.tensor_tensor(
                        dst, dst, pe_b, op=ADD,
                    )._wait_ge(t_sem, i + g).then_inc(v_sem, 1)
                else:
                    o_sl = obuf[:, s * D:(s + 1) * D]
                    nc.vector.tensor_tensor(
                        o_sl, o_sl, pe_sl(j), op=ADD,
                    )._wait_ge(t_sem, i + 1).then_inc(v_sem, 1)
                for z in range(g):
                    if i + la + z < ND:
                        emit_ts(i + la + z)
                i += g

        # ---------------- PE stream ----------------
        @block.tensor
        def _(tensor):
            tensor.wait_ge(w_sem, 1)
            for w in range(5):
                nc.tensor.matmul(psa[:, 0:512], warm[:, 0:P], warm[:],
                                 start=True, stop=True)
            tensor.wait_ge(s_c, 32)
            tensor.wait_ge(s_mh, 16)
            waited_p = waited_b = False
            for k, j in enumerate(pe_list):
                t = j % T_TILES
                bank = (k % 4) * PSW
                q = k // 2
                lhsT = c2m[0:2, k * P:(k + 1) * P]
                if k == n_head and c2mr_d is not None:
                    tensor.wait_ge(s_m, 16)
                if t == 0 and not waited_p:
                    tensor.wait_ge(s_p, 16)
                    waited_p = True
                if t == 1 and not waited_b:
                    tensor.wait_ge(s_b, 16)
                    waited_b = True
                mm = nc.tensor.matmul(psa[:, bank:bank + 512], lhsT,
                                      c2v[:, 0:512], start=True, stop=False)
                if k >= 4:
                    mm._wait_ge(c_sem, q - 1)
                nc.tensor.matmul(psa[:, bank + 512:bank + D], lhsT,
                                 c2v[:, 512:D], start=True, stop=False)
                nc.tensor.matmul(psa[:, bank:bank + 512], idm,
                                 pe_sl(j)[:, 0:512], start=False, stop=True)
                mm4 = nc.tensor.matmul(psa[:, bank + 512:bank + D], idm,
                                       pe_sl(j)[:, 512:D], start=False, stop=True)
                if k % 2 == 1 or k == N_PE - 1:
                    mm4.then_inc(p_sem, 1)

        # ---------------- Pool: c2m rest + out-DMAs ----------------
        @block.gpsimd
        def _(gpsimd):
            if c2mr_d is not None:
                gpsimd.dma_start(c2m[:, NH:], c2mr_d[:]).then_inc(s_m, 16)
            work = []
            for k, j in enumerate(pe_list):
                q = k // 2
                if q == n_pairs - 1:
                    continue  # whole last pair drained by SP
                est = 3600 + (q + 1) * 1480
                work.append((est, "a", k, j))
            for i, j in enumerate(dve_tiles):
                if dve_on_pool(i):
                    work.append((2400 + (i + 1) * 720, "d", i, j))
            for est, kind, idx, j in sorted(work):
                if kind == "a":
                    s = aslot(idx)
                    q = idx // 2
                    fin = s_finp if q == n_pairs - 1 else sa[idx % ns_a]
                    gpsimd.dma_start(out_d[j * P:(j + 1) * P, :],
                                     obuf[:, s * D:(s + 1) * D]
                                     )._wait_ge(c_sem, q + 1).then_inc(fin, 16)
                else:
                    s = dslot(idx)
                    fin = sd[s] if idx < NR else s_finp
                    gpsimd.dma_start(out_d[j * P:(j + 1) * P, :],
                                     obuf[:, s * D:(s + 1) * D]
                                     )._wait_ge(v_sem, vneed[idx]).then_inc(fin, 16)

    nc.finalize()
    return nc, dve_tiles, pe_list


def _choose_pe_tiles(tok_cols):
    """Even-sized PE-stream tile set containing every tok column."""
    pe = sorted(tok_cols)
    for j in range(0, J, 2):          # prefer t=0 tiles (j even)
        if len(pe) >= N_PE_DEFAULT and len(pe) % 2 == 0:
            break
        if j not in tok_cols:
            pe.append(j)
    if len(pe) % 2:                   # pad to even with any spare tile
        for j in range(J):
            if j not in pe:
                pe.append(j)
                break
    return tuple(sorted(pe))


def _prepare(inputs):
    ids = np.asarray(inputs["input_ids"])
    seg = np.asarray(inputs["segment_label"])
    W_tok = np.asarray(inputs["W_tok"], dtype=np.float32)
    b_tok = np.asarray(inputs["b_tok"], dtype=np.float32)
    W_seg = np.asarray(inputs["W_seg"], dtype=np.float32)
    b_seg = np.asarray(inputs["b_seg"], dtype=np.float32)
    pe = np.asarray(inputs["pe"], dtype=np.float32).reshape(SEQ, D)

    c_tok = (W_tok[:, 0] + b_tok).astype(np.float32)
    c_seg = (W_seg[:, 0] + b_seg).astype(np.float32)
    m1_full = (ids == 0).astype(np.float32)
    m2_full = (seg == 0).astype(np.float32)

    per_core = []
    tok_cols = set()
    for c in range(N_CORES):
        sl = slice(c * S_SH, (c + 1) * S_SH)
        # [B, S_SH] -> [P, J]: column j = b*T_TILES + t, partition p
        m1 = m1_full[:, sl].reshape(B, T_TILES, P).transpose(2, 0, 1).reshape(P, J)
        m2 = m2_full[:, sl].reshape(B, T_TILES, P).transpose(2, 0, 1).reshape(P, J)
        pe_sl = pe[sl].reshape(T_TILES, P, D)
        tok_cols.update(np.nonzero(m1.any(axis=0))[0].tolist())
        per_core.append((pe_sl, m1, m2))

    pe_tiles = _choose_pe_tiles(tok_cols)
    _, pe_list = _order_tiles(pe_tiles)
    n_pe = len(pe_list)
    n_head = min(4, max(n_pe, 1))

    in_maps = []
    for pe_sl, m1, m2 in per_core:
        c16 = np.zeros((P, C16), np.float16)
        c16[:, CSEG_O:CSEG_O + D] = c_seg[None, :].astype(np.float16)
        c16[:, PE0_O:PE0_O + D] = pe_sl[0].astype(np.float16)
        c16[:, PE1_O:PE1_O + D] = pe_sl[1].astype(np.float16)
        c16[:, IDM_O:IDM_O + P] = np.eye(P, dtype=np.float16)
        cmv = m2.astype(np.float32)
        c2v = np.zeros((2, D), np.float16)
        c2v[0] = c_seg.astype(np.float16)
        c2v[1] = c_tok.astype(np.float16)
        c2m = np.zeros((2, max(n_pe, 1) * P), np.float16)
        for k, j in enumerate(pe_list):
            c2m[0, k * P:(k + 1) * P] = m2[:, j].astype(np.float16)
            c2m[1, k * P:(k + 1) * P] = m1[:, j].astype(np.float16)
        m = {"c16": c16, "cm": cmv, "c2v": c2v, "c2mh": c2m[:, :n_head * P]}
        if n_pe > n_head:
            m["c2mr"] = c2m[:, n_head * P:]
        in_maps.append(m)
    return in_maps, pe_tiles


def kernel(**inputs) -> np.ndarray:
    global LAST_RESULTS
    in_maps, pe_tiles = _prepare(inputs)
    key = ("v2", pe_tiles)
    if key not in _prog_cache:
        _prog_cache[key] = _build(pe_tiles)[0]
    nc = _prog_cache[key]

    trace = bool(int(os.environ.get("BASS_KERNEL_TRACE", "0")))
    try:
        res = run_bass_kernel_spmd(
            nc, in_maps, list(range(N_CORES)), trace=trace,
            trace_cores=list(range(N_CORES)) if trace else None,
        )
    except ModuleNotFoundError:
        # axon builds without the NTFF profile hook crash when tracing is
        # requested; degrade to an untraced run.
        os.environ["BASS_NEVER_TRACE"] = "1"
        res = run_bass_kernel_spmd(nc, in_maps, list(range(N_CORES)), trace=False)
    LAST_RESULTS = res

    out = np.empty((B, SEQ, D), dtype=np.float32)
    for c in range(N_CORES):
        out[:, c * S_SH:(c + 1) * S_SH, :] = (
            np.asarray(res.results[c]["out"])
            .astype(np.float32).reshape(B, S_SH, D)
        )
    return out
